# revision 17
# baseline (speedup 1.0000x reference)
"""Causal self-attention with RoPE on 8 trn2 NeuronCores (Bass/Tile).

Sharding: tensor-parallel over heads (4 heads/core) x data-parallel over
batch (B=2). Core i = b*4 + t handles batch b, heads 4t..4t+3.

Per-core dataflow (all matmuls bf16, fp32 PSUM):
  - host passes x.T [C, T] so contractions always have K on partitions
  - qk.T [512(j), T] = w_qk.T @ x.T   (lhsT = w_qk natural [c, j])
  - RoPE on q.T/k.T in [d, t] layout; per-head d-columns are permuted to
    [evens | odds] so the rotation is 32-partition-block aligned.
    Fused: uw[:, {0,1}, :] = ps * (cos|sin) in one DVE op (free-dim
    broadcast of ps), then 32-row swap of uw[:,1] via 4 DMAs, then a
    bf16 add.
  - v [T, 256] natural (lhsT = x.T tile) + ones column (aug) per head
  - S.T [k, q] per head = (k.T).T @ q.T   (K = d = 64; heads of a pair run
    on disjoint PE row groups -> concurrent). Diagonal k-tiles skip the
    fully-masked column prefix (matmul N = 512-off).
  - P = exp(0.125 * S.T) on ScalarE; causal masking only multiplies the
    128x128 triangle block of each diagonal k-tile (tri mask); the
    fully-masked prefix columns are never computed nor consumed.
  - y_aug.T [65, q] = v_aug.T @ P  accumulated over k tiles (AV also
    skips masked prefix columns); row 64 is the softmax denominator.
  - y.T = y.T * recip(denom)  (denoms batched, broadcast via DRAM)
  - out_partial.T [C, q] = w_proj_shard.T @ y.T ; host sums the 4 partials
    of each batch and transposes.

Schedule: segment(0) runs first; attention for chunk qc interleaves
segment(qc+1) and proj(qc-1) blocks as PE filler, so ScalarE exp starts
as early as possible and the TensorE stream stays dense (HAM-warm).
Input DMAs are chunked (xT per (ci, chunk)) and spread across queues so
segment(0) is unblocked within a few us of kernel start.
"""

import numpy as np
import ml_dtypes

B, T, C, H = 2, 2048, 1024, 16
HD = C // H          # 64
HPC = H // 4         # heads per core = 4
JQK = 2 * HPC * HD   # 512  (q|k columns per core)
JV = HPC * HD        # 256
N_CORES = 8
TC = 512             # q/t chunk (moving free dim)
NQC = T // TC        # 4 q-chunks
NKT = T // 128       # 16 k-tiles
NCT = C // 128       # 8 contraction tiles
VGW = 66             # v group width per head: 64 v cols + 1 ones + 1 pad
ST_G = 2             # score psum group (k-tiles per exp)

_CACHE = {}


def _build():
    import concourse.bass as bass
    import concourse.tile as tile
    from concourse import bacc, mybir
    EXP = mybir.ActivationFunctionType.Exp

    bf16 = mybir.dt.bfloat16
    f32 = mybir.dt.float32

    nc = bacc.Bacc("TRN2", target_bir_lowering=False, debug=False,
                   num_devices=N_CORES)
    xT = nc.dram_tensor("xT", [C, T], bf16, kind="ExternalInput").ap()
    w_qk = nc.dram_tensor("w_qk", [C, JQK], bf16, kind="ExternalInput").ap()
    w_v = nc.dram_tensor("w_v", [C, JV], bf16, kind="ExternalInput").ap()
    w_pr = nc.dram_tensor("w_pr", [JV, C], bf16, kind="ExternalInput").ap()
    cs = nc.dram_tensor("cs", [64, 2 * T], bf16, kind="ExternalInput").ap()
    tri = nc.dram_tensor("tri", [128, 128], bf16, kind="ExternalInput").ap()
    yT_out = nc.dram_tensor("yT", [C, T], bf16, kind="ExternalOutput").ap()

    with tile.TileContext(nc) as tc:
        import contextlib
        ctx = contextlib.ExitStack()
        with ctx:
            const = ctx.enter_context(tc.tile_pool(name="const", bufs=1))
            ppool = ctx.enter_context(tc.tile_pool(name="p", bufs=4))
            ypool = ctx.enter_context(tc.tile_pool(name="ysb", bufs=4))
            rpool = ctx.enter_context(tc.tile_pool(name="r", bufs=4))
            npool = ctx.enter_context(tc.tile_pool(name="n", bufs=2))
            # PSUM: shared pool 3 slots x 2 banks + 2 y accumulators x 1
            # bank = 8 banks
            mm_ps = ctx.enter_context(
                tc.tile_pool(name="mmps", bufs=3, space="PSUM"))
            y_ps = ctx.enter_context(
                tc.tile_pool(name="yps", bufs=2, space="PSUM"))

            # ---- resident inputs.  segment(0) needs w_qk + xT chunk 0
            # (+ w_v); issue those first, spread over queues, with the
            # rest of xT / cs / w_pr behind them. ----
            t_xT = const.tile([128, NCT, T], bf16)
            t_wqk = const.tile([128, NCT, JQK], bf16)
            t_wv = const.tile([128, NCT, JV], bf16)
            t_cs = const.tile([128, 2, T], bf16)
            t_tri = const.tile([128, 128], bf16)
            t_wpr = const.tile([128, 2, C], bf16)

            # queue plan: gpsimd carries w_qk/cs/tri then stays light for
            # the latency-critical rope-swap DMAs; sync carries the bulk
            # xT/w_v/w_pr stream; ScalarE issues nothing (it is the exp
            # engine and its queue must not stall activations).
            qs = [nc.sync, nc.gpsimd]
            for ci in range(NCT):
                nc.gpsimd.dma_start(out=t_wqk[:, ci, :],
                                    in_=w_qk[ci * 128:(ci + 1) * 128, :])
            for ci in range(NCT):
                nc.sync.dma_start(out=t_xT[:, ci, 0:TC],
                                  in_=xT[ci * 128:(ci + 1) * 128, 0:TC])
            for k in range(2):
                nc.gpsimd.dma_start(out=t_cs[0:64, k, :],
                                    in_=cs[:, k * T:(k + 1) * T])
            nc.gpsimd.dma_start(out=t_tri, in_=tri)
            # replicate cos|sin rows 0-63 to 64-127 on-chip (saves HBM)
            nc.gpsimd.dma_start(out=t_cs[64:128, :, :], in_=t_cs[0:64, :, :])
            for ci in range(NCT):
                nc.sync.dma_start(out=t_wv[:, ci, :],
                                  in_=w_v[ci * 128:(ci + 1) * 128, :])
            for tcn in range(1, 4):
                sl = slice(tcn * TC, (tcn + 1) * TC)
                for ci in range(NCT):
                    nc.sync.dma_start(
                        out=t_xT[:, ci, sl],
                        in_=xT[ci * 128:(ci + 1) * 128, sl])
            for ci in range(2):
                nc.sync.dma_start(out=t_wpr[:, ci, :],
                                  in_=w_pr[ci * 128:(ci + 1) * 128, :])
            # all-ones stationary column for the denominator broadcast
            # matmul (rb = ones.T @ recip_row)
            t_one = const.tile([1, 64], f32)
            nc.vector.memset(t_one, 1.0)

            # qk.T buffer: [128, jt, T]; jt 0..1 = q head-pairs, 2..3 = k
            t_qkT = const.tile([128, 4, T], bf16)

            # v buffer: [128(t), kt_hi, 4*66]; per head 64 v + ones + pad
            t_v = const.tile([128, NKT, 4 * VGW], bf16)
            vv = t_v.rearrange("p k (h c) -> p k h c", h=4)
            for h in range(4):
                nc.vector.memset(vv[:, :, h, 64:65], 1.0)

            def segment_blocks(tcn):
                """yield per-block callables: 4 qk(+rope) blocks then 4
                v blocks for t-chunk tcn."""
                sl = slice(tcn * TC, (tcn + 1) * TC)

                def qk_block(jt, sl=sl, tcn=tcn):
                    ps = mm_ps.tile([128, TC], f32, tag="mm", name="psqk")
                    for ci in range(NCT):
                        nc.tensor.matmul(
                            ps,
                            lhsT=t_wqk[:, ci, jt * 128:(jt + 1) * 128],
                            rhs=t_xT[:, ci, sl],
                            start=(ci == 0), stop=(ci == NCT - 1))
                    q = t_qkT[:, jt, sl]
                    # RoPE (even/odd-split): o_e = e*cos - o*sin,
                    # o_o = o*cos + e*sin; cs rows carry the sign pattern.
                    # One fused mul (ps broadcast over the cos|sin dim),
                    # then a 32-row block swap via DMA (DVE can't permute
                    # partitions), then a bf16 add.
                    uw = rpool.tile([128, 2, TC], bf16, tag="ruw")
                    ws = rpool.tile([128, TC], bf16, tag="rws")
                    nc.vector.tensor_mul(
                        out=uw, in0=ps.unsqueeze(1).broadcast_to((128, 2, TC)),
                        in1=t_cs[:, :, sl])
                    for b0 in range(0, 128, 32):
                        nc.gpsimd.dma_start(
                            out=ws[b0:b0 + 32, :],
                            in_=uw[b0 ^ 32:(b0 ^ 32) + 32, 1, :])
                    nc.vector.tensor_add(out=q, in0=uw[:, 0, :], in1=ws)

                def v_block(tt):
                    ps = mm_ps.tile([128, JV], f32, tag="mm", name="psv")
                    for ci in range(NCT):
                        nc.tensor.matmul(
                            ps,
                            lhsT=t_xT[:, ci, tt * 128:(tt + 1) * 128],
                            rhs=t_wv[:, ci, :],
                            start=(ci == 0), stop=(ci == NCT - 1))
                    nc.vector.tensor_copy(
                        out=vv[:, tt, :, 0:64],
                        in_=ps.rearrange("p (h c) -> p h c", h=4))

                for jt in range(4):
                    yield (lambda j=jt: qk_block(j))
                for tt in range(4 * tcn, 4 * tcn + 4):
                    yield (lambda t=tt: v_block(t))

            def segment(tcn):
                for f in segment_blocks(tcn):
                    f()

            def attn_half(qc, hp, y_qc, filler=None):
                """scores+softmax+AV+normalize for q-chunk qc, head pair
                hp. AV for group g is emitted after ST of group g+2 (its
                exp has landed by then) so the PE never head-of-line
                blocks on an exp wait; `filler` supplies extra PE work
                (segments/proj) to densify the stream."""
                nkt = 4 * (qc + 1)
                qsl = slice(qc * TC, (qc + 1) * TC)
                pA = ppool.tile([128, NKT, TC], bf16, tag="pbuf")
                pB = ppool.tile([128, NKT, TC], bf16, tag="pbuf")
                yA = y_ps.tile([65, TC], f32, tag="yps")
                yB = y_ps.tile([65, TC], f32, tag="yps")

                def off_of(kt):
                    off = kt * 128 - qc * TC
                    return off if 0 <= off < TC else 0

                def av(kt):
                    off = off_of(kt)
                    for half, (yps, p) in enumerate(((yA, pA), (yB, pB))):
                        h = 2 * hp + half
                        nc.tensor.matmul(
                            yps[:, off:TC],
                            lhsT=t_v[:, kt, h * VGW:h * VGW + 65],
                            rhs=p[:, kt, off:TC],
                            start=(kt == 0), stop=(kt == nkt - 1))

                ngrp = (nkt + ST_G - 1) // ST_G
                for g in range(ngrp):
                    g0 = g * ST_G
                    gl = min(ST_G, nkt - g0)
                    stA = mm_ps.tile([128, ST_G, TC], f32, tag="mm")
                    stB = mm_ps.tile([128, ST_G, TC], f32, tag="mm")
                    for kg in range(gl):
                        kt = g0 + kg
                        ksl = slice(kt * 128, (kt + 1) * 128)
                        off = off_of(kt)
                        qsl_o = slice(qc * TC + off, (qc + 1) * TC)
                        nc.tensor.matmul(
                            stA[:, kg, off:TC],
                            lhsT=t_qkT[0:64, 2 + hp, ksl],
                            rhs=t_qkT[0:64, hp, qsl_o],
                            start=True, stop=True)
                        nc.tensor.matmul(
                            stB[:, kg, off:TC],
                            lhsT=t_qkT[64:128, 2 + hp, ksl],
                            rhs=t_qkT[64:128, hp, qsl_o],
                            start=True, stop=True)
                    # AV for the group 2 back, plus periodic filler to
                    # cover the PE-vs-ACT deficit
                    if g >= 2:
                        for kt in range((g - 2) * ST_G,
                                        (g - 2) * ST_G + ST_G):
                            av(kt)
                    if filler is not None and (g < 2 or g % 3 == 2):
                        f = next(filler, None)
                        if f is not None:
                            f()
                    nc.scalar.activation(
                        out=pA[:, g0:g0 + gl, :], in_=stA[:, 0:gl, :],
                        func=EXP, scale=0.125)
                    nc.scalar.activation(
                        out=pB[:, g0:g0 + gl, :], in_=stB[:, 0:gl, :],
                        func=EXP, scale=0.125)
                    # causal mask: only the 128-wide triangle block of
                    # diagonal k-tiles needs masking (prefix cols are
                    # skipped in the AV/score matmuls entirely)
                    for kg in range(gl):
                        kt = g0 + kg
                        off = kt * 128 - qc * TC
                        if 0 <= off < TC:
                            dsl = slice(off, off + 128)
                            nc.vector.tensor_mul(
                                out=pA[:, kt, dsl], in0=pA[:, kt, dsl],
                                in1=t_tri)
                            nc.vector.tensor_mul(
                                out=pB[:, kt, dsl], in0=pB[:, kt, dsl],
                                in1=t_tri)
                for kt in range(max(0, (ngrp - 2) * ST_G), nkt):
                    av(kt)
                # normalize: approx-reciprocal of the 2 denominator rows,
                # broadcast across 64 partitions via a K=1 matmul with an
                # all-ones stationary column (no DRAM round-trip), scale
                d2 = npool.tile([1, 2, TC], f32, tag="d2")
                r2 = npool.tile([1, 2, TC], f32, tag="r2")
                nc.vector.tensor_copy(out=d2[:, 0, :], in_=yA[64:65, :])
                nc.vector.tensor_copy(out=d2[:, 1, :], in_=yB[64:65, :])
                nc.vector.reciprocal_approx_fast(out=r2, in_=d2)
                for half, yps in ((0, yA), (1, yB)):
                    rb_ps = mm_ps.tile([64, TC], f32, tag="mm", name="psrb")
                    nc.tensor.matmul(rb_ps, lhsT=t_one,
                                     rhs=r2[:, half, :],
                                     start=True, stop=True)
                    rb = rpool.tile([64, TC], f32, tag="r64")
                    nc.vector.tensor_copy(out=rb, in_=rb_ps)
                    nc.vector.tensor_mul(
                        out=y_qc[half * 64:(half + 1) * 64, hp, :],
                        in0=yps[0:64, :], in1=rb)

            def proj_blocks(qc, y_qc, final=False):
                for co in range(NCT):
                    def co_block(co=co):
                        ps = mm_ps.tile([128, TC], f32, tag="mm", name="psp")
                        for ci in range(2):
                            nc.tensor.matmul(
                                ps,
                                lhsT=t_wpr[:, ci, co * 128:(co + 1) * 128],
                                rhs=y_qc[:, ci, :],
                                start=(ci == 0), stop=(ci == 1))
                        o_sb = rpool.tile([128, TC], bf16, tag="osb")
                        # in the final chunk ScalarE is done with exp, so
                        # split the PSUM->SBUF casts across both engines
                        # to shorten the tail
                        if final and co % 2 == 1:
                            nc.scalar.copy(out=o_sb, in_=ps)
                        else:
                            nc.vector.tensor_copy(out=o_sb, in_=ps)
                        qs[co % 2].dma_start(
                            out=yT_out[co * 128:(co + 1) * 128,
                                       qc * TC:(qc + 1) * TC],
                            in_=o_sb)
                    yield co_block

            # interleave: segment(0), then attention chunk qc consumes
            # segment(qc+1) and proj(qc-1) blocks as PE filler while
            # ScalarE works through the exp volume
            y_qcs = [None] * NQC
            import itertools
            segment(0)
            for qc in range(NQC):
                y_qc = ypool.tile([128, 2, TC], bf16, tag="yqc")
                y_qcs[qc] = y_qc
                fill = iter(())
                if qc + 1 < NQC:
                    fill = itertools.chain(fill, segment_blocks(qc + 1))
                if qc >= 1:
                    fill = itertools.chain(fill, proj_blocks(qc - 1,
                                                            y_qcs[qc - 1]))
                attn_half(qc, 0, y_qc, fill)
                attn_half(qc, 1, y_qc, fill)
                for f in fill:
                    f()
            for f in proj_blocks(NQC - 1, y_qcs[NQC - 1], final=True):
                f()

    nc.compile()
    return nc


def _prep_inputs(x, w_qkv, w_proj, freqs_cos, freqs_sin):
    bf = ml_dtypes.bfloat16
    cos = np.asarray(freqs_cos, np.float32)   # [T, 32]
    sin = np.asarray(freqs_sin, np.float32)
    # even/odd-split RoPE: within each head, q/k columns are permuted to
    # [d0,d2,..,d62, d1,d3,..,d63]; patterns are 32-row blocks
    cos_p = np.tile(cos.T, (2, 1))                             # [64, T]
    sin_p = np.concatenate([sin.T, -sin.T], 0)                 # [64, T]
    cs = np.concatenate([cos_p, sin_p], axis=1).astype(bf)     # [64, 2T]
    eo = np.concatenate([np.arange(0, HD, 2), np.arange(1, HD, 2)])
    # causal triangle for the 128-wide diagonal block: keep iff col >= row
    kp = np.arange(128)
    tri = (kp[None, :] >= kp[:, None]).astype(bf)   # [row k, col j]: j >= k

    x = np.asarray(x, np.float32)
    w_qkv = np.asarray(w_qkv, np.float32)
    w_proj = np.asarray(w_proj, np.float32)
    in_maps = []
    # per-head even/odd column permutation for q and k blocks
    perm = np.concatenate([h * HD + eo for h in range(H)])
    wq_p = w_qkv[:, 0 * C:1 * C][:, perm]
    wk_p = w_qkv[:, 1 * C:2 * C][:, perm]
    for i in range(N_CORES):
        b, t = divmod(i, 4)
        jq = slice(t * JV, (t + 1) * JV)
        wq = wq_p[:, jq]
        wk = wk_p[:, jq]
        wv = w_qkv[:, 2 * C:3 * C][:, jq]
        in_maps.append({
            "xT": np.ascontiguousarray(x[b].T).astype(bf),
            "w_qk": np.concatenate([wq, wk], axis=1).astype(bf),
            "w_v": np.ascontiguousarray(wv).astype(bf),
            "w_pr": np.ascontiguousarray(w_proj[t * JV:(t + 1) * JV, :]).astype(bf),
            "cs": cs, "tri": tri,
        })
    return in_maps


def run(inputs, trace=False):
    from concourse import bass_utils
    if "nc" not in _CACHE:
        _CACHE["nc"] = _build()
    nc = _CACHE["nc"]
    in_maps = _prep_inputs(**inputs)
    res = bass_utils.run_bass_kernel_spmd(
        nc, in_maps, core_ids=list(range(N_CORES)), trace=trace)
    out = np.empty((B, T, C), np.float32)
    for b in range(B):
        acc = res.results[b * 4]["yT"].astype(np.float32)
        for t in range(1, 4):
            acc += res.results[b * 4 + t]["yT"]
        out[b] = acc.T
    return out, res


def kernel(**inputs):
    out, _ = run(inputs, trace=False)
    return out


# revision 27
# speedup vs baseline: 1.0560x; 1.0560x over previous
"""Causal self-attention with RoPE on 8 trn2 NeuronCores (Bass/Tile).

Sharding: tensor-parallel over heads (4 heads/core) x data-parallel over
batch (B=2). Core i = b*4 + t handles batch b, heads 4t..4t+3.

Per-core dataflow (all matmuls bf16, fp32 PSUM):
  - host passes x.T [C, T] so contractions always have K on partitions
  - qk.T [512(j), T] = w_qk.T @ x.T   (lhsT = w_qk natural [c, j])
  - RoPE on q.T/k.T in [d, t] layout; per-head d-columns are permuted to
    [evens | odds] so the rotation is 32-partition-block aligned.
    Fused: uw[:, {0,1}, :] = ps * (cos|sin) in one DVE op (free-dim
    broadcast of ps), then 32-row swap of uw[:,1] via 4 DMAs, then a
    bf16 add.
  - v [T, 256] natural (lhsT = x.T tile) + ones column (aug) per head
  - S.T [k, q] per head = (k.T).T @ q.T   (K = d = 64; heads of a pair run
    on disjoint PE row groups -> concurrent). Diagonal k-tiles skip the
    fully-masked column prefix (matmul N = 512-off).
  - P = exp(0.125 * S.T) on ScalarE; causal masking only multiplies the
    128x128 triangle block of each diagonal k-tile (tri mask); the
    fully-masked prefix columns are never computed nor consumed.
  - y_aug.T [65, q] = v_aug.T @ P  accumulated over k tiles (AV also
    skips masked prefix columns); row 64 is the softmax denominator.
  - y.T = y.T * recip(denom)  (denoms batched, broadcast via DRAM)
  - out_partial.T [C, q] = w_proj_shard.T @ y.T ; host sums the 4 partials
    of each batch and transposes.

Schedule: segment(0) runs first; attention for chunk qc interleaves
segment(qc+1) and proj(qc-1) blocks as PE filler, so ScalarE exp starts
as early as possible and the TensorE stream stays dense (HAM-warm).
Input DMAs are chunked (xT per (ci, chunk)) and spread across queues so
segment(0) is unblocked within a few us of kernel start.
"""

import numpy as np
import ml_dtypes

B, T, C, H = 2, 2048, 1024, 16
HD = C // H          # 64
HPC = H // 4         # heads per core = 4
JQK = 2 * HPC * HD   # 512  (q|k columns per core)
JV = HPC * HD        # 256
N_CORES = 8
TC = 512             # q/t chunk (moving free dim)
NQC = T // TC        # 4 q-chunks
NKT = T // 128       # 16 k-tiles
NCT = C // 128       # 8 contraction tiles
VGW = 66             # v group width per head: 64 v cols + 1 ones + 1 pad
ST_G = 2             # score psum group (k-tiles per exp)

_CACHE = {}


def _build():
    import concourse.bass as bass
    import concourse.tile as tile
    from concourse import bacc, mybir
    EXP = mybir.ActivationFunctionType.Exp

    bf16 = mybir.dt.bfloat16
    f32 = mybir.dt.float32

    nc = bacc.Bacc("TRN2", target_bir_lowering=False, debug=False,
                   num_devices=N_CORES)
    xT = nc.dram_tensor("xT", [C, T], bf16, kind="ExternalInput").ap()
    w_qk = nc.dram_tensor("w_qk", [C, JQK], bf16, kind="ExternalInput").ap()
    w_v = nc.dram_tensor("w_v", [C, JV], bf16, kind="ExternalInput").ap()
    w_pr = nc.dram_tensor("w_pr", [JV, C], bf16, kind="ExternalInput").ap()
    cs = nc.dram_tensor("cs", [128, 2 * T], bf16, kind="ExternalInput").ap()
    tri = nc.dram_tensor("tri", [128, 128], bf16, kind="ExternalInput").ap()
    yT_out = nc.dram_tensor("yT", [C, T], bf16, kind="ExternalOutput").ap()

    with tile.TileContext(nc) as tc:
        import contextlib
        ctx = contextlib.ExitStack()
        with ctx:
            const = ctx.enter_context(tc.tile_pool(name="const", bufs=1))
            ppool = ctx.enter_context(tc.tile_pool(name="p", bufs=4))
            ypool = ctx.enter_context(tc.tile_pool(name="ysb", bufs=4))
            rpool = ctx.enter_context(tc.tile_pool(name="r", bufs=4))
            npool = ctx.enter_context(tc.tile_pool(name="n", bufs=2))
            # PSUM: shared pool 3 slots x 2 banks + 2 y accumulators x 1
            # bank = 8 banks
            mm_ps = ctx.enter_context(
                tc.tile_pool(name="mmps", bufs=3, space="PSUM"))
            y_ps = ctx.enter_context(
                tc.tile_pool(name="yps", bufs=2, space="PSUM"))

            # ---- resident inputs.  segment(0) needs w_qk + xT chunk 0
            # (+ w_v); issue those first, spread over queues, with the
            # rest of xT / cs / w_pr behind them. ----
            t_xT = const.tile([128, NCT, T], bf16)
            t_wqk = const.tile([128, NCT, JQK], bf16)
            t_wv = const.tile([128, NCT, JV], bf16)
            t_cs = const.tile([128, 2, T], bf16)
            t_tri = const.tile([128, 128], bf16)
            t_wpr = const.tile([128, 2, C], bf16)

            # queue plan: gpsimd carries w_qk/tri/cs then stays light for
            # the latency-critical rope-swap DMAs (cs is loaded one chunk
            # at a time, prefetched per segment, so swaps never queue
            # behind bulk); sync carries the bulk xT/w_v/w_pr stream;
            # ScalarE issues nothing (it is the exp engine and its queue
            # must not stall activations).
            qs = [nc.sync, nc.gpsimd]

            def cs_chunk(tcn):
                sl = slice(tcn * TC, (tcn + 1) * TC)
                for k in range(2):
                    nc.gpsimd.dma_start(
                        out=t_cs[:, k, sl],
                        in_=cs[:, k * T + tcn * TC:k * T + (tcn + 1) * TC])

            for ci in range(NCT):
                nc.gpsimd.dma_start(out=t_wqk[:, ci, :],
                                    in_=w_qk[ci * 128:(ci + 1) * 128, :])
            nc.gpsimd.dma_start(out=t_tri, in_=tri)
            cs_chunk(0)
            for ci in range(NCT):
                nc.sync.dma_start(out=t_xT[:, ci, 0:TC],
                                  in_=xT[ci * 128:(ci + 1) * 128, 0:TC])
            for ci in range(NCT):
                nc.sync.dma_start(out=t_wv[:, ci, :],
                                  in_=w_v[ci * 128:(ci + 1) * 128, :])
            for ci in range(NCT):
                nc.sync.dma_start(out=t_xT[:, ci, TC:2 * TC],
                                  in_=xT[ci * 128:(ci + 1) * 128, TC:2 * TC])
            for ci in range(2):
                nc.sync.dma_start(out=t_wpr[:, ci, :],
                                  in_=w_pr[ci * 128:(ci + 1) * 128, :])
            for tcn in range(2, 4):
                sl = slice(tcn * TC, (tcn + 1) * TC)
                for ci in range(NCT):
                    nc.sync.dma_start(
                        out=t_xT[:, ci, sl],
                        in_=xT[ci * 128:(ci + 1) * 128, sl])
            # all-ones stationary column for the denominator broadcast
            # matmul (rb = ones.T @ recip_row)
            t_one = const.tile([1, 64], f32)
            nc.vector.memset(t_one, 1.0)

            # qk.T buffer: [128, jt, T]; jt 0..1 = q head-pairs, 2..3 = k
            t_qkT = const.tile([128, 4, T], bf16)

            # v buffer: [128(t), kt_hi, 4*66]; per head 64 v + ones + pad
            t_v = const.tile([128, NKT, 4 * VGW], bf16)
            vv = t_v.rearrange("p k (h c) -> p k h c", h=4)
            for h in range(4):
                nc.vector.memset(vv[:, :, h, 64:65], 1.0)

            def segment_blocks(tcn):
                """yield per-block callables: 4 qk(+rope) blocks then 4
                v blocks for t-chunk tcn."""
                sl = slice(tcn * TC, (tcn + 1) * TC)

                def qk_block(jt, sl=sl, tcn=tcn):
                    ps = mm_ps.tile([128, TC], f32, tag="mm", name="psqk")
                    for ci in range(NCT):
                        nc.tensor.matmul(
                            ps,
                            lhsT=t_wqk[:, ci, jt * 128:(jt + 1) * 128],
                            rhs=t_xT[:, ci, sl],
                            start=(ci == 0), stop=(ci == NCT - 1))
                    q = t_qkT[:, jt, sl]
                    # RoPE (even/odd-split): o_e = e*cos - o*sin,
                    # o_o = o*cos + e*sin; cs rows carry the sign pattern.
                    # One fused mul (ps broadcast over the cos|sin dim),
                    # then a 32-row block swap via DMA (DVE can't permute
                    # partitions), then a bf16 add.
                    uw = rpool.tile([128, 2, TC], bf16, tag="ruw")
                    ws = rpool.tile([128, TC], bf16, tag="rws")
                    nc.vector.tensor_mul(
                        out=uw, in0=ps.unsqueeze(1).broadcast_to((128, 2, TC)),
                        in1=t_cs[:, :, sl])
                    for b0 in range(0, 128, 32):
                        nc.gpsimd.dma_start(
                            out=ws[b0:b0 + 32, :],
                            in_=uw[b0 ^ 32:(b0 ^ 32) + 32, 1, :])
                    nc.vector.tensor_add(out=q, in0=uw[:, 0, :], in1=ws)

                def v_block(tt):
                    ps = mm_ps.tile([128, JV], f32, tag="mm", name="psv")
                    for ci in range(NCT):
                        nc.tensor.matmul(
                            ps,
                            lhsT=t_xT[:, ci, tt * 128:(tt + 1) * 128],
                            rhs=t_wv[:, ci, :],
                            start=(ci == 0), stop=(ci == NCT - 1))
                    nc.vector.tensor_copy(
                        out=vv[:, tt, :, 0:64],
                        in_=ps.rearrange("p (h c) -> p h c", h=4))

                # prefetch next chunk's cos/sin while this one computes;
                # qk order [0,2,1,3] completes head-pair 0's q AND k
                # first so its scores (and exp) start one rope earlier
                if tcn + 1 < NQC:
                    yield (lambda t=tcn + 1: cs_chunk(t))
                for jt in (0, 2, 1, 3):
                    yield (lambda j=jt: qk_block(j))
                for tt in range(4 * tcn, 4 * tcn + 4):
                    yield (lambda t=tt: v_block(t))

            def segment(tcn):
                for f in segment_blocks(tcn):
                    f()

            def attn_half(qc, hp, y_qc, filler=None, hook=None):
                """scores+softmax+AV for q-chunk qc, head pair hp. AV for
                group g is emitted after ST of group g+2 (its exp has
                landed by then) so the PE never head-of-line blocks on an
                exp wait; `filler` supplies extra PE work (segments/proj)
                to densify the stream. The normalize step is NOT emitted
                here: it is returned as a closure and run via `hook`
                inside the NEXT attn_half, so its broadcast matmul never
                stalls the PE FIFO on the DVE reciprocal chain (the PSUM
                accumulators are evacuated to SBUF immediately)."""
                nkt = 4 * (qc + 1)
                qsl = slice(qc * TC, (qc + 1) * TC)
                pA = ppool.tile([128, NKT, TC], bf16, tag="pbuf")
                pB = ppool.tile([128, NKT, TC], bf16, tag="pbuf")
                yA = y_ps.tile([65, TC], f32, tag="yps")
                yB = y_ps.tile([65, TC], f32, tag="yps")

                def off_of(kt):
                    off = kt * 128 - qc * TC
                    return off if 0 <= off < TC else 0

                def av(kt):
                    off = off_of(kt)
                    for half, (yps, p) in enumerate(((yA, pA), (yB, pB))):
                        h = 2 * hp + half
                        nc.tensor.matmul(
                            yps[:, off:TC],
                            lhsT=t_v[:, kt, h * VGW:h * VGW + 65],
                            rhs=p[:, kt, off:TC],
                            start=(kt == 0), stop=(kt == nkt - 1))

                ngrp = (nkt + ST_G - 1) // ST_G
                for g in range(ngrp):
                    g0 = g * ST_G
                    gl = min(ST_G, nkt - g0)
                    stA = mm_ps.tile([128, ST_G, TC], f32, tag="mm")
                    stB = mm_ps.tile([128, ST_G, TC], f32, tag="mm")
                    for kg in range(gl):
                        kt = g0 + kg
                        ksl = slice(kt * 128, (kt + 1) * 128)
                        off = off_of(kt)
                        qsl_o = slice(qc * TC + off, (qc + 1) * TC)
                        nc.tensor.matmul(
                            stA[:, kg, off:TC],
                            lhsT=t_qkT[0:64, 2 + hp, ksl],
                            rhs=t_qkT[0:64, hp, qsl_o],
                            start=True, stop=True)
                        nc.tensor.matmul(
                            stB[:, kg, off:TC],
                            lhsT=t_qkT[64:128, 2 + hp, ksl],
                            rhs=t_qkT[64:128, hp, qsl_o],
                            start=True, stop=True)
                    # AV for the group 2 back, plus periodic filler to
                    # cover the PE-vs-ACT deficit
                    if g >= 2:
                        for kt in range((g - 2) * ST_G,
                                        (g - 2) * ST_G + ST_G):
                            av(kt)
                    # the previous half's deferred normalize must precede
                    # any filler (proj fillers read the y_qc it writes)
                    if g == 0 and hook is not None:
                        hook()
                        hook = None
                    if filler is not None and (g < 2 or g % 3 == 2):
                        f = next(filler, None)
                        if f is not None:
                            f()
                    nc.scalar.activation(
                        out=pA[:, g0:g0 + gl, :], in_=stA[:, 0:gl, :],
                        func=EXP, scale=0.125)
                    nc.scalar.activation(
                        out=pB[:, g0:g0 + gl, :], in_=stB[:, 0:gl, :],
                        func=EXP, scale=0.125)
                    # causal mask: only the 128-wide triangle block of
                    # diagonal k-tiles needs masking (prefix cols are
                    # skipped in the AV/score matmuls entirely)
                    for kg in range(gl):
                        kt = g0 + kg
                        off = kt * 128 - qc * TC
                        if 0 <= off < TC:
                            dsl = slice(off, off + 128)
                            nc.vector.tensor_mul(
                                out=pA[:, kt, dsl], in0=pA[:, kt, dsl],
                                in1=t_tri)
                            nc.vector.tensor_mul(
                                out=pB[:, kt, dsl], in0=pB[:, kt, dsl],
                                in1=t_tri)
                for kt in range(max(0, (ngrp - 2) * ST_G), nkt):
                    av(kt)
                # evacuate PSUM now: denominators to f32 rows, bodies to
                # bf16 SBUF; approx-reciprocal runs right behind on DVE
                d2 = npool.tile([1, 2, TC], f32, tag="d2")
                r2 = npool.tile([1, 2, TC], f32, tag="r2")
                ySb = npool.tile([64, 2, TC], bf16, tag="ysb")
                nc.vector.tensor_copy(out=d2[:, 0, :], in_=yA[64:65, :])
                nc.vector.tensor_copy(out=d2[:, 1, :], in_=yB[64:65, :])
                nc.vector.tensor_copy(out=ySb[:, 0, :], in_=yA[0:64, :])
                nc.vector.tensor_copy(out=ySb[:, 1, :], in_=yB[0:64, :])
                nc.vector.reciprocal_approx_fast(out=r2, in_=d2)

                def finalize():
                    # broadcast recip across 64 partitions via a K=1
                    # matmul with an all-ones stationary column (no DRAM
                    # round-trip), then scale into y_qc
                    for half in (0, 1):
                        rb_ps = mm_ps.tile([64, TC], f32, tag="mm",
                                           name="psrb")
                        nc.tensor.matmul(rb_ps, lhsT=t_one,
                                         rhs=r2[:, half, :],
                                         start=True, stop=True)
                        rb = rpool.tile([64, TC], bf16, tag="r64")
                        nc.vector.tensor_copy(out=rb, in_=rb_ps)
                        nc.vector.tensor_mul(
                            out=y_qc[half * 64:(half + 1) * 64, hp, :],
                            in0=ySb[:, half, :], in1=rb)
                return finalize

            def proj_blocks(qc, y_qc, final=False):
                for co in range(NCT):
                    def co_block(co=co):
                        ps = mm_ps.tile([128, TC], f32, tag="mm", name="psp")
                        for ci in range(2):
                            nc.tensor.matmul(
                                ps,
                                lhsT=t_wpr[:, ci, co * 128:(co + 1) * 128],
                                rhs=y_qc[:, ci, :],
                                start=(ci == 0), stop=(ci == 1))
                        o_sb = rpool.tile([128, TC], bf16, tag="osb")
                        # in the final chunk ScalarE is done with exp, so
                        # split the PSUM->SBUF casts across both engines
                        # to shorten the tail
                        if final and co % 2 == 1:
                            nc.scalar.copy(out=o_sb, in_=ps)
                        else:
                            nc.vector.tensor_copy(out=o_sb, in_=ps)
                        qs[co % 2].dma_start(
                            out=yT_out[co * 128:(co + 1) * 128,
                                       qc * TC:(qc + 1) * TC],
                            in_=o_sb)
                    yield co_block

            # interleave: segment(0), then attention chunk qc consumes
            # segment(qc+1) and proj(qc-1) blocks as PE filler while
            # ScalarE works through the exp volume
            y_qcs = [None] * NQC
            import itertools
            segment(0)
            fin = None
            for qc in range(NQC):
                y_qc = ypool.tile([128, 2, TC], bf16, tag="yqc")
                y_qcs[qc] = y_qc
                fill = iter(())
                if qc + 1 < NQC:
                    fill = itertools.chain(fill, segment_blocks(qc + 1))
                if qc >= 1:
                    fill = itertools.chain(fill, proj_blocks(qc - 1,
                                                            y_qcs[qc - 1]))
                fin = attn_half(qc, 0, y_qc, fill, hook=fin)
                fin = attn_half(qc, 1, y_qc, fill, hook=fin)
                for f in fill:
                    f()
            fin()
            for f in proj_blocks(NQC - 1, y_qcs[NQC - 1], final=True):
                f()

    nc.compile()
    return nc


def _prep_inputs(x, w_qkv, w_proj, freqs_cos, freqs_sin):
    bf = ml_dtypes.bfloat16
    cos = np.asarray(freqs_cos, np.float32)   # [T, 32]
    sin = np.asarray(freqs_sin, np.float32)
    # even/odd-split RoPE: within each head, q/k columns are permuted to
    # [d0,d2,..,d62, d1,d3,..,d63]; patterns are 32-row blocks
    cos_p = np.tile(cos.T, (4, 1))                             # [128, T]
    sin_p = np.tile(np.concatenate([sin.T, -sin.T], 0), (2, 1))
    cs = np.concatenate([cos_p, sin_p], axis=1).astype(bf)     # [128, 2T]
    eo = np.concatenate([np.arange(0, HD, 2), np.arange(1, HD, 2)])
    # causal triangle for the 128-wide diagonal block: keep iff col >= row
    kp = np.arange(128)
    tri = (kp[None, :] >= kp[:, None]).astype(bf)   # [row k, col j]: j >= k

    x = np.asarray(x, np.float32)
    w_qkv = np.asarray(w_qkv, np.float32)
    w_proj = np.asarray(w_proj, np.float32)
    in_maps = []
    # per-head even/odd column permutation for q and k blocks
    perm = np.concatenate([h * HD + eo for h in range(H)])
    wq_p = w_qkv[:, 0 * C:1 * C][:, perm]
    wk_p = w_qkv[:, 1 * C:2 * C][:, perm]
    for i in range(N_CORES):
        b, t = divmod(i, 4)
        jq = slice(t * JV, (t + 1) * JV)
        wq = wq_p[:, jq]
        wk = wk_p[:, jq]
        wv = w_qkv[:, 2 * C:3 * C][:, jq]
        in_maps.append({
            "xT": np.ascontiguousarray(x[b].T).astype(bf),
            "w_qk": np.concatenate([wq, wk], axis=1).astype(bf),
            "w_v": np.ascontiguousarray(wv).astype(bf),
            "w_pr": np.ascontiguousarray(w_proj[t * JV:(t + 1) * JV, :]).astype(bf),
            "cs": cs, "tri": tri,
        })
    return in_maps


def run(inputs, trace=False):
    from concourse import bass_utils
    if "nc" not in _CACHE:
        _CACHE["nc"] = _build()
    nc = _CACHE["nc"]
    in_maps = _prep_inputs(**inputs)
    res = bass_utils.run_bass_kernel_spmd(
        nc, in_maps, core_ids=list(range(N_CORES)), trace=trace)
    out = np.empty((B, T, C), np.float32)
    for b in range(B):
        acc = res.results[b * 4]["yT"].astype(np.float32)
        for t in range(1, 4):
            acc += res.results[b * 4 + t]["yT"]
        out[b] = acc.T
    return out, res


def kernel(**inputs):
    out, _ = run(inputs, trace=False)
    return out


# revision 33
# speedup vs baseline: 1.2664x; 1.1993x over previous
"""Causal self-attention with RoPE on 8 trn2 NeuronCores (Bass/Tile).

Sharding: tensor-parallel over heads (4 heads/core) x data-parallel over
batch (B=2). Core i = b*4 + t handles batch b, heads 4t..4t+3.

Per-core dataflow (all matmuls bf16, fp32 PSUM):
  - host passes x.T [C, T] so contractions always have K on partitions
  - qk.T [512(j), T] = w_qk.T @ x.T   (lhsT = w_qk natural [c, j])
  - RoPE on q.T/k.T in [d, t] layout; per-head d-columns are permuted to
    [evens | odds] so the rotation is 32-partition-block aligned.
    Fused: uw[:, {0,1}, :] = ps * (cos|sin) in one DVE op (free-dim
    broadcast of ps), then 32-row swap of uw[:,1] via 4 DMAs, then a
    bf16 add.
  - v [T, 256] natural (lhsT = x.T tile) + ones column (aug) per head
  - S.T [k, q] per head = (k.T).T @ q.T   (K = d = 64; heads of a pair run
    on disjoint PE row groups -> concurrent). Diagonal k-tiles skip the
    fully-masked column prefix (matmul N = 512-off).
  - P = exp(0.125 * S.T) on ScalarE; causal masking only multiplies the
    128x128 triangle block of each diagonal k-tile (tri mask); the
    fully-masked prefix columns are never computed nor consumed.
  - y_aug.T [65, q] = v_aug.T @ P  accumulated over k tiles (AV also
    skips masked prefix columns); row 64 is the softmax denominator.
  - y.T = y.T * recip(denom)  (denoms batched, broadcast via DRAM)
  - out_partial.T [C, q] = w_proj_shard.T @ y.T ; host sums the 4 partials
    of each batch and transposes.

Schedule: segment(0) runs first; attention for chunk qc interleaves
segment(qc+1) and proj(qc-1) blocks as PE filler, so ScalarE exp starts
as early as possible and the TensorE stream stays dense (HAM-warm).
Input DMAs are chunked (xT per (ci, chunk)) and spread across queues so
segment(0) is unblocked within a few us of kernel start.
"""

import numpy as np
import ml_dtypes

B, T, C, H = 2, 2048, 1024, 16
HD = C // H          # 64
HPC = H // 4         # heads per core = 4
JQK = 2 * HPC * HD   # 512  (q|k columns per core)
JV = HPC * HD        # 256
N_CORES = 8
TC = 512             # q/t chunk (moving free dim)
NQC = T // TC        # 4 q-chunks
NKT = T // 128       # 16 k-tiles
NCT = C // 128       # 8 contraction tiles
VGW = 66             # v group width per head: 64 v cols + 1 ones + 1 pad
ST_G = 2             # score psum group (k-tiles per exp)

_CACHE = {}


def _build():
    import concourse.bass as bass
    import concourse.tile as tile
    from concourse import bacc, mybir
    EXP = mybir.ActivationFunctionType.Exp

    bf16 = mybir.dt.bfloat16
    f32 = mybir.dt.float32

    nc = bacc.Bacc("TRN2", target_bir_lowering=False, debug=False,
                   num_devices=N_CORES)
    xT = nc.dram_tensor("xT", [C, T], bf16, kind="ExternalInput").ap()
    w_qk = nc.dram_tensor("w_qk", [C, JQK], bf16, kind="ExternalInput").ap()
    w_v = nc.dram_tensor("w_v", [C, JV], bf16, kind="ExternalInput").ap()
    w_pr = nc.dram_tensor("w_pr", [JV, C], bf16, kind="ExternalInput").ap()
    cs = nc.dram_tensor("cs", [128, 2 * T], bf16, kind="ExternalInput").ap()
    tri = nc.dram_tensor("tri", [128, 128], bf16, kind="ExternalInput").ap()
    yT_out = nc.dram_tensor("yT", [C, T], bf16, kind="ExternalOutput").ap()

    with tile.TileContext(nc) as tc:
        import contextlib
        ctx = contextlib.ExitStack()
        with ctx:
            const = ctx.enter_context(tc.tile_pool(name="const", bufs=1))
            ppool = ctx.enter_context(tc.tile_pool(name="p", bufs=4))
            ypool = ctx.enter_context(tc.tile_pool(name="ysb", bufs=4))
            rpool = ctx.enter_context(tc.tile_pool(name="r", bufs=4))
            npool = ctx.enter_context(tc.tile_pool(name="n", bufs=2))
            # PSUM: shared pool 3 slots x 2 banks + 2 y accumulators x 1
            # bank = 8 banks
            mm_ps = ctx.enter_context(
                tc.tile_pool(name="mmps", bufs=3, space="PSUM"))
            y_ps = ctx.enter_context(
                tc.tile_pool(name="yps", bufs=2, space="PSUM"))
            dram = ctx.enter_context(
                tc.tile_pool(name="dram", bufs=4, space="DRAM"))

            # ---- resident inputs.  segment(0) needs w_qk + xT chunk 0
            # (+ w_v); issue those first, spread over queues, with the
            # rest of xT / cs / w_pr behind them. ----
            t_xT = const.tile([128, NCT, T], bf16)
            t_wqk = const.tile([128, NCT, JQK], bf16)
            t_wv = const.tile([128, NCT, JV], bf16)
            t_cs = const.tile([128, 2, T], bf16)
            t_tri = const.tile([128, 128], bf16)
            t_wpr = const.tile([128, 2, C], bf16)

            # queue plan: gpsimd carries w_qk/tri/cs then stays light for
            # the latency-critical rope-swap DMAs (cs is loaded one chunk
            # at a time, prefetched per segment, so swaps never queue
            # behind bulk); sync carries the bulk xT/w_v/w_pr stream;
            # ScalarE issues nothing (it is the exp engine and its queue
            # must not stall activations).
            qs = [nc.sync, nc.gpsimd]

            def cs_chunk(tcn):
                sl = slice(tcn * TC, (tcn + 1) * TC)
                for k in range(2):
                    nc.gpsimd.dma_start(
                        out=t_cs[:, k, sl],
                        in_=cs[:, k * T + tcn * TC:k * T + (tcn + 1) * TC])

            for ci in range(NCT):
                nc.gpsimd.dma_start(out=t_wqk[:, ci, :],
                                    in_=w_qk[ci * 128:(ci + 1) * 128, :])
            nc.gpsimd.dma_start(out=t_tri, in_=tri)
            cs_chunk(0)
            for ci in range(NCT):
                nc.sync.dma_start(out=t_xT[:, ci, 0:TC],
                                  in_=xT[ci * 128:(ci + 1) * 128, 0:TC])
            for ci in range(NCT):
                nc.sync.dma_start(out=t_wv[:, ci, :],
                                  in_=w_v[ci * 128:(ci + 1) * 128, :])
            for ci in range(NCT):
                nc.sync.dma_start(out=t_xT[:, ci, TC:2 * TC],
                                  in_=xT[ci * 128:(ci + 1) * 128, TC:2 * TC])
            for ci in range(2):
                nc.sync.dma_start(out=t_wpr[:, ci, :],
                                  in_=w_pr[ci * 128:(ci + 1) * 128, :])
            for tcn in range(2, 4):
                sl = slice(tcn * TC, (tcn + 1) * TC)
                for ci in range(NCT):
                    nc.sync.dma_start(
                        out=t_xT[:, ci, sl],
                        in_=xT[ci * 128:(ci + 1) * 128, sl])
            # all-ones stationary column for the denominator broadcast
            # matmul (rb = ones.T @ recip_row)
            t_one = const.tile([1, 64], f32)
            nc.vector.memset(t_one, 1.0)

            # qk.T buffer: [128, jt, T]; jt 0..1 = q head-pairs, 2..3 = k
            t_qkT = const.tile([128, 4, T], bf16)

            # v buffer: [128(t), kt_hi, 4*66]; per head 64 v + ones + pad
            t_v = const.tile([128, NKT, 4 * VGW], bf16)
            vv = t_v.rearrange("p k (h c) -> p k h c", h=4)
            for h in range(4):
                nc.vector.memset(vv[:, :, h, 64:65], 1.0)

            def segment_blocks(tcn):
                """yield per-block callables: 4 qk(+rope) blocks then 4
                v blocks for t-chunk tcn."""
                sl = slice(tcn * TC, (tcn + 1) * TC)

                def qk_block(jt, sl=sl, tcn=tcn):
                    ps = mm_ps.tile([128, TC], f32, tag="mm", name="psqk")
                    for ci in range(NCT):
                        nc.tensor.matmul(
                            ps,
                            lhsT=t_wqk[:, ci, jt * 128:(jt + 1) * 128],
                            rhs=t_xT[:, ci, sl],
                            start=(ci == 0), stop=(ci == NCT - 1))
                    q = t_qkT[:, jt, sl]
                    # RoPE (even/odd-split): o_e = e*cos - o*sin,
                    # o_o = o*cos + e*sin; cs rows carry the sign pattern.
                    # One fused mul (ps broadcast over the cos|sin dim),
                    # then a 32-row block swap via DMA (DVE can't permute
                    # partitions), then a bf16 add.
                    uw = rpool.tile([128, 2, TC], bf16, tag="ruw")
                    ws = rpool.tile([128, TC], bf16, tag="rws")
                    nc.vector.tensor_mul(
                        out=uw, in0=ps.unsqueeze(1).broadcast_to((128, 2, TC)),
                        in1=t_cs[:, :, sl])
                    for b0 in range(0, 128, 32):
                        nc.gpsimd.dma_start(
                            out=ws[b0:b0 + 32, :],
                            in_=uw[b0 ^ 32:(b0 ^ 32) + 32, 1, :])
                    nc.vector.tensor_add(out=q, in0=uw[:, 0, :], in1=ws)

                def v_block(tt):
                    ps = mm_ps.tile([128, JV], f32, tag="mm", name="psv")
                    for ci in range(NCT):
                        nc.tensor.matmul(
                            ps,
                            lhsT=t_xT[:, ci, tt * 128:(tt + 1) * 128],
                            rhs=t_wv[:, ci, :],
                            start=(ci == 0), stop=(ci == NCT - 1))
                    nc.vector.tensor_copy(
                        out=vv[:, tt, :, 0:64],
                        in_=ps.rearrange("p (h c) -> p h c", h=4))

                # prefetch next chunk's cos/sin while this one computes;
                # qk order [0,2,1,3] completes head-pair 0's q AND k
                # first so its scores (and exp) start one rope earlier
                if tcn + 1 < NQC:
                    yield (lambda t=tcn + 1: cs_chunk(t))
                for jt in (0, 2, 1, 3):
                    yield (lambda j=jt: qk_block(j))
                for tt in range(4 * tcn, 4 * tcn + 4):
                    yield (lambda t=tt: v_block(t))

            def segment(tcn):
                for f in segment_blocks(tcn):
                    f()

            def attn_half(qc, hp, y_qc, filler=None, final=False):
                """scores+softmax+AV+normalize for q-chunk qc, head pair
                hp. AV for group g is emitted after ST of group g+2 (its
                exp has landed by then) so the PE never head-of-line
                blocks on an exp wait; `filler` supplies extra PE work
                (segments/proj) to densify the stream. The normalize uses
                a PE-free DRAM-broadcast (so the PE FIFO never waits on
                the DVE reciprocal); only the final half uses a broadcast
                matmul (PE is idle then) to cut the tail latency."""
                nkt = 4 * (qc + 1)
                qsl = slice(qc * TC, (qc + 1) * TC)
                pA = ppool.tile([128, NKT, TC], bf16, tag="pbuf")
                pB = ppool.tile([128, NKT, TC], bf16, tag="pbuf")
                yA = y_ps.tile([65, TC], f32, tag="yps")
                yB = y_ps.tile([65, TC], f32, tag="yps")

                def off_of(kt):
                    off = kt * 128 - qc * TC
                    return off if 0 <= off < TC else 0

                def av(kt):
                    off = off_of(kt)
                    for half, (yps, p) in enumerate(((yA, pA), (yB, pB))):
                        h = 2 * hp + half
                        nc.tensor.matmul(
                            yps[:, off:TC],
                            lhsT=t_v[:, kt, h * VGW:h * VGW + 65],
                            rhs=p[:, kt, off:TC],
                            start=(kt == 0), stop=(kt == nkt - 1))

                ngrp = (nkt + ST_G - 1) // ST_G
                for g in range(ngrp):
                    g0 = g * ST_G
                    gl = min(ST_G, nkt - g0)
                    stA = mm_ps.tile([128, ST_G, TC], f32, tag="mm")
                    stB = mm_ps.tile([128, ST_G, TC], f32, tag="mm")
                    for kg in range(gl):
                        kt = g0 + kg
                        ksl = slice(kt * 128, (kt + 1) * 128)
                        off = off_of(kt)
                        qsl_o = slice(qc * TC + off, (qc + 1) * TC)
                        nc.tensor.matmul(
                            stA[:, kg, off:TC],
                            lhsT=t_qkT[0:64, 2 + hp, ksl],
                            rhs=t_qkT[0:64, hp, qsl_o],
                            start=True, stop=True)
                        nc.tensor.matmul(
                            stB[:, kg, off:TC],
                            lhsT=t_qkT[64:128, 2 + hp, ksl],
                            rhs=t_qkT[64:128, hp, qsl_o],
                            start=True, stop=True)
                    # AV for the group 2 back, plus periodic filler to
                    # cover the PE-vs-ACT deficit
                    if g >= 2:
                        for kt in range((g - 2) * ST_G,
                                        (g - 2) * ST_G + ST_G):
                            av(kt)
                    if filler is not None and (g < 2 or g % 3 == 2):
                        f = next(filler, None)
                        if f is not None:
                            f()
                    nc.scalar.activation(
                        out=pA[:, g0:g0 + gl, :], in_=stA[:, 0:gl, :],
                        func=EXP, scale=0.125)
                    nc.scalar.activation(
                        out=pB[:, g0:g0 + gl, :], in_=stB[:, 0:gl, :],
                        func=EXP, scale=0.125)
                    # causal mask: only the 128-wide triangle block of
                    # diagonal k-tiles needs masking (prefix cols are
                    # skipped in the AV/score matmuls entirely)
                    for kg in range(gl):
                        kt = g0 + kg
                        off = kt * 128 - qc * TC
                        if 0 <= off < TC:
                            dsl = slice(off, off + 128)
                            nc.vector.tensor_mul(
                                out=pA[:, kt, dsl], in0=pA[:, kt, dsl],
                                in1=t_tri)
                            nc.vector.tensor_mul(
                                out=pB[:, kt, dsl], in0=pB[:, kt, dsl],
                                in1=t_tri)
                for kt in range(max(0, (ngrp - 2) * ST_G), nkt):
                    av(kt)
                # evacuate PSUM now: denominators to f32 rows, bodies to
                # bf16 SBUF; approx-reciprocal runs right behind on DVE
                d2 = npool.tile([1, 2, TC], f32, tag="d2")
                r2 = npool.tile([1, 2, TC], f32, tag="r2")
                ySb = npool.tile([64, 2, TC], bf16, tag="ysb")
                nc.vector.tensor_copy(out=d2[:, 0, :], in_=yA[64:65, :])
                nc.vector.tensor_copy(out=d2[:, 1, :], in_=yB[64:65, :])
                nc.vector.tensor_copy(out=ySb[:, 0, :], in_=yA[0:64, :])
                nc.vector.tensor_copy(out=ySb[:, 1, :], in_=yB[0:64, :])
                nc.vector.reciprocal_approx_fast(out=r2, in_=d2)
                if final:
                    for half in (0, 1):
                        rb_ps = mm_ps.tile([64, TC], f32, tag="mm",
                                           name="psrb")
                        nc.tensor.matmul(rb_ps, lhsT=t_one,
                                         rhs=r2[:, half, :],
                                         start=True, stop=True)
                        rb = rpool.tile([64, TC], f32, tag="r64")
                        nc.vector.tensor_copy(out=rb, in_=rb_ps)
                        nc.vector.tensor_mul(
                            out=y_qc[half * 64:(half + 1) * 64, hp, :],
                            in0=ySb[:, half, :], in1=rb)
                else:
                    rd = dram.tile([1, 2, TC], f32, tag="rd")
                    nc.gpsimd.dma_start(out=rd, in_=r2)
                    for half in (0, 1):
                        rb = rpool.tile([64, TC], f32, tag="r64")
                        nc.gpsimd.dma_start(
                            out=rb,
                            in_=rd[:, half, :].to_broadcast((64, TC)))
                        nc.vector.tensor_mul(
                            out=y_qc[half * 64:(half + 1) * 64, hp, :],
                            in0=ySb[:, half, :], in1=rb)

            def proj_blocks(qc, y_qc, final=False):
                for co in range(NCT):
                    def co_block(co=co):
                        ps = mm_ps.tile([128, TC], f32, tag="mm", name="psp")
                        for ci in range(2):
                            nc.tensor.matmul(
                                ps,
                                lhsT=t_wpr[:, ci, co * 128:(co + 1) * 128],
                                rhs=y_qc[:, ci, :],
                                start=(ci == 0), stop=(ci == 1))
                        o_sb = rpool.tile([128, TC], bf16, tag="osb")
                        # in the final chunk ScalarE is done with exp, so
                        # split the PSUM->SBUF casts across both engines
                        # to shorten the tail
                        if final and co % 2 == 1:
                            nc.scalar.copy(out=o_sb, in_=ps)
                        else:
                            nc.vector.tensor_copy(out=o_sb, in_=ps)
                        qs[co % 2].dma_start(
                            out=yT_out[co * 128:(co + 1) * 128,
                                       qc * TC:(qc + 1) * TC],
                            in_=o_sb)
                    yield co_block

            # interleave: segment(0), then attention chunk qc consumes
            # segment(qc+1) and proj(qc-1) blocks as PE filler while
            # ScalarE works through the exp volume
            y_qcs = [None] * NQC
            import itertools
            segment(0)
            for qc in range(NQC):
                y_qc = ypool.tile([128, 2, TC], bf16, tag="yqc")
                y_qcs[qc] = y_qc
                fill = iter(())
                if qc + 1 < NQC:
                    fill = itertools.chain(fill, segment_blocks(qc + 1))
                if qc >= 1:
                    fill = itertools.chain(fill, proj_blocks(qc - 1,
                                                            y_qcs[qc - 1]))
                attn_half(qc, 0, y_qc, fill)
                attn_half(qc, 1, y_qc, fill, final=(qc == NQC - 1))
                for f in fill:
                    f()
            for f in proj_blocks(NQC - 1, y_qcs[NQC - 1], final=True):
                f()

    nc.compile()
    return nc


def _prep_inputs(x, w_qkv, w_proj, freqs_cos, freqs_sin):
    bf = ml_dtypes.bfloat16
    cos = np.asarray(freqs_cos, np.float32)   # [T, 32]
    sin = np.asarray(freqs_sin, np.float32)
    # even/odd-split RoPE: within each head, q/k columns are permuted to
    # [d0,d2,..,d62, d1,d3,..,d63]; patterns are 32-row blocks
    cos_p = np.tile(cos.T, (4, 1))                             # [128, T]
    sin_p = np.tile(np.concatenate([sin.T, -sin.T], 0), (2, 1))
    cs = np.concatenate([cos_p, sin_p], axis=1).astype(bf)     # [128, 2T]
    eo = np.concatenate([np.arange(0, HD, 2), np.arange(1, HD, 2)])
    # causal triangle for the 128-wide diagonal block: keep iff col >= row
    kp = np.arange(128)
    tri = (kp[None, :] >= kp[:, None]).astype(bf)   # [row k, col j]: j >= k

    x = np.asarray(x, np.float32)
    w_qkv = np.asarray(w_qkv, np.float32)
    w_proj = np.asarray(w_proj, np.float32)
    in_maps = []
    # per-head even/odd column permutation for q and k blocks
    perm = np.concatenate([h * HD + eo for h in range(H)])
    wq_p = w_qkv[:, 0 * C:1 * C][:, perm]
    wk_p = w_qkv[:, 1 * C:2 * C][:, perm]
    for i in range(N_CORES):
        b, t = divmod(i, 4)
        jq = slice(t * JV, (t + 1) * JV)
        wq = wq_p[:, jq]
        wk = wk_p[:, jq]
        wv = w_qkv[:, 2 * C:3 * C][:, jq]
        in_maps.append({
            "xT": np.ascontiguousarray(x[b].T).astype(bf),
            "w_qk": np.concatenate([wq, wk], axis=1).astype(bf),
            "w_v": np.ascontiguousarray(wv).astype(bf),
            "w_pr": np.ascontiguousarray(w_proj[t * JV:(t + 1) * JV, :]).astype(bf),
            "cs": cs, "tri": tri,
        })
    return in_maps


def run(inputs, trace=False):
    from concourse import bass_utils
    if "nc" not in _CACHE:
        _CACHE["nc"] = _build()
    nc = _CACHE["nc"]
    in_maps = _prep_inputs(**inputs)
    res = bass_utils.run_bass_kernel_spmd(
        nc, in_maps, core_ids=list(range(N_CORES)), trace=trace)
    out = np.empty((B, T, C), np.float32)
    for b in range(B):
        acc = res.results[b * 4]["yT"].astype(np.float32)
        for t in range(1, 4):
            acc += res.results[b * 4 + t]["yT"]
        out[b] = acc.T
    return out, res


def kernel(**inputs):
    out, _ = run(inputs, trace=False)
    return out


# revision 37
# speedup vs baseline: 1.3178x; 1.0406x over previous
"""Causal self-attention with RoPE on 8 trn2 NeuronCores (Bass/Tile).

Sharding: tensor-parallel over heads (4 heads/core) x data-parallel over
batch (B=2). Core i = b*4 + t handles batch b, heads 4t..4t+3.

Per-core dataflow (all matmuls bf16, fp32 PSUM):
  - host passes x.T [C, T] so contractions always have K on partitions
  - qk.T [512(j), T] = w_qk.T @ x.T   (lhsT = w_qk natural [c, j])
  - RoPE on q.T/k.T in [d, t] layout; per-head d-columns are permuted to
    [evens | odds] so the rotation is 32-partition-block aligned.
    Fused: uw[:, {0,1}, :] = ps * (cos|sin) in one DVE op (free-dim
    broadcast of ps), then 32-row swap of uw[:,1] via 4 DMAs, then a
    bf16 add.
  - v [T, 256] natural (lhsT = x.T tile) + ones column (aug) per head
  - S.T [k, q] per head = (k.T).T @ q.T   (K = d = 64; heads of a pair run
    on disjoint PE row groups -> concurrent). Diagonal k-tiles skip the
    fully-masked column prefix (matmul N = 512-off).
  - P = exp(0.125 * S.T) on ScalarE; causal masking only multiplies the
    128x128 triangle block of each diagonal k-tile (tri mask); the
    fully-masked prefix columns are never computed nor consumed.
  - y_aug.T [65, q] = v_aug.T @ P  accumulated over k tiles (AV also
    skips masked prefix columns); row 64 is the softmax denominator.
  - y.T = y.T * recip(denom)  (denoms batched, broadcast via DRAM)
  - out_partial.T [C, q] = w_proj_shard.T @ y.T ; host sums the 4 partials
    of each batch and transposes.

Schedule: segment(0) runs first; attention for chunk qc interleaves
segment(qc+1) and proj(qc-1) blocks as PE filler, so ScalarE exp starts
as early as possible and the TensorE stream stays dense (HAM-warm).
Input DMAs are chunked (xT per (ci, chunk)) and spread across queues so
segment(0) is unblocked within a few us of kernel start.
"""

import numpy as np
import ml_dtypes

B, T, C, H = 2, 2048, 1024, 16
HD = C // H          # 64
HPC = H // 4         # heads per core = 4
JQK = 2 * HPC * HD   # 512  (q|k columns per core)
JV = HPC * HD        # 256
N_CORES = 8
TC = 512             # q/t chunk (moving free dim)
NQC = T // TC        # 4 q-chunks
NKT = T // 128       # 16 k-tiles
NCT = C // 128       # 8 contraction tiles
VGW = 66             # v group width per head: 64 v cols + 1 ones + 1 pad
ST_G = 2             # score psum group (k-tiles per exp)

_CACHE = {}


def _build():
    import concourse.bass as bass
    import concourse.tile as tile
    from concourse import bacc, mybir
    EXP = mybir.ActivationFunctionType.Exp

    bf16 = mybir.dt.bfloat16
    f32 = mybir.dt.float32

    nc = bacc.Bacc("TRN2", target_bir_lowering=False, debug=False,
                   num_devices=N_CORES)
    xT = nc.dram_tensor("xT", [C, T], bf16, kind="ExternalInput").ap()
    w_qk = nc.dram_tensor("w_qk", [C, JQK], bf16, kind="ExternalInput").ap()
    w_v = nc.dram_tensor("w_v", [C, JV], bf16, kind="ExternalInput").ap()
    w_pr = nc.dram_tensor("w_pr", [JV, C], bf16, kind="ExternalInput").ap()
    cs = nc.dram_tensor("cs", [128, 2 * T], bf16, kind="ExternalInput").ap()
    tri = nc.dram_tensor("tri", [128, 128], bf16, kind="ExternalInput").ap()
    yT_out = nc.dram_tensor("yT", [C, T], bf16, kind="ExternalOutput").ap()

    with tile.TileContext(nc) as tc:
        import contextlib
        ctx = contextlib.ExitStack()
        with ctx:
            const = ctx.enter_context(tc.tile_pool(name="const", bufs=1))
            ppool = ctx.enter_context(tc.tile_pool(name="p", bufs=4))
            ypool = ctx.enter_context(tc.tile_pool(name="ysb", bufs=4))
            rpool = ctx.enter_context(tc.tile_pool(name="r", bufs=4))
            npool = ctx.enter_context(tc.tile_pool(name="n", bufs=2))
            # PSUM: shared pool 3 slots x 2 banks + 2 y accumulators x 1
            # bank = 8 banks
            mm_ps = ctx.enter_context(
                tc.tile_pool(name="mmps", bufs=3, space="PSUM"))
            y_ps = ctx.enter_context(
                tc.tile_pool(name="yps", bufs=2, space="PSUM"))
            dram = ctx.enter_context(
                tc.tile_pool(name="dram", bufs=4, space="DRAM"))

            # ---- resident inputs.  segment(0) needs w_qk + xT chunk 0
            # (+ w_v); issue those first, spread over queues, with the
            # rest of xT / cs / w_pr behind them. ----
            t_xT = const.tile([128, NCT, T], bf16)
            t_wqk = const.tile([128, NCT, JQK], bf16)
            t_wv = const.tile([128, NCT, JV], bf16)
            t_cs = const.tile([128, 2, T], bf16)
            t_tri = const.tile([128, 128], bf16)
            t_wpr = const.tile([128, 2, C], bf16)

            # queue plan: gpsimd carries w_qk/tri/cs then stays light for
            # the latency-critical rope-swap DMAs (cs is loaded one chunk
            # at a time, prefetched per segment, so swaps never queue
            # behind bulk); sync carries the bulk xT/w_v/w_pr stream;
            # ScalarE issues nothing (it is the exp engine and its queue
            # must not stall activations).
            qs = [nc.sync, nc.gpsimd]

            def cs_chunk(tcn):
                sl = slice(tcn * TC, (tcn + 1) * TC)
                for k in range(2):
                    nc.gpsimd.dma_start(
                        out=t_cs[:, k, sl],
                        in_=cs[:, k * T + tcn * TC:k * T + (tcn + 1) * TC])

            for ci in range(NCT):
                nc.gpsimd.dma_start(out=t_wqk[:, ci, :],
                                    in_=w_qk[ci * 128:(ci + 1) * 128, :])
            nc.gpsimd.dma_start(out=t_tri, in_=tri)
            cs_chunk(0)
            for ci in range(NCT):
                nc.sync.dma_start(out=t_xT[:, ci, 0:TC],
                                  in_=xT[ci * 128:(ci + 1) * 128, 0:TC])
            for ci in range(NCT):
                nc.sync.dma_start(out=t_wv[:, ci, :],
                                  in_=w_v[ci * 128:(ci + 1) * 128, :])
            for ci in range(NCT):
                nc.sync.dma_start(out=t_xT[:, ci, TC:2 * TC],
                                  in_=xT[ci * 128:(ci + 1) * 128, TC:2 * TC])
            for ci in range(2):
                nc.sync.dma_start(out=t_wpr[:, ci, :],
                                  in_=w_pr[ci * 128:(ci + 1) * 128, :])
            for tcn in range(2, 4):
                sl = slice(tcn * TC, (tcn + 1) * TC)
                for ci in range(NCT):
                    nc.sync.dma_start(
                        out=t_xT[:, ci, sl],
                        in_=xT[ci * 128:(ci + 1) * 128, sl])
            # all-ones stationary column for the denominator broadcast
            # matmul (rb = ones.T @ recip_row)
            t_one = const.tile([1, 64], f32)
            nc.vector.memset(t_one, 1.0)

            # qk.T buffer: [128, jt, T]; jt 0..1 = q head-pairs, 2..3 = k
            t_qkT = const.tile([128, 4, T], bf16)

            # v buffer: [128(t), kt_hi, 4*66]; per head 64 v + ones + pad
            t_v = const.tile([128, NKT, 4 * VGW], bf16)
            vv = t_v.rearrange("p k (h c) -> p k h c", h=4)
            for h in range(4):
                nc.vector.memset(vv[:, :, h, 64:65], 1.0)

            def segment_blocks(tcn):
                """yield per-block callables: 4 qk(+rope) blocks then 4
                v blocks for t-chunk tcn."""
                sl = slice(tcn * TC, (tcn + 1) * TC)

                def qk_block(jt, sl=sl, tcn=tcn):
                    ps = mm_ps.tile([128, TC], f32, tag="mm", name="psqk")
                    for ci in range(NCT):
                        nc.tensor.matmul(
                            ps,
                            lhsT=t_wqk[:, ci, jt * 128:(jt + 1) * 128],
                            rhs=t_xT[:, ci, sl],
                            start=(ci == 0), stop=(ci == NCT - 1))
                    q = t_qkT[:, jt, sl]
                    # RoPE (even/odd-split): o_e = e*cos - o*sin,
                    # o_o = o*cos + e*sin; cs rows carry the sign pattern.
                    # One fused mul (ps broadcast over the cos|sin dim),
                    # then a 32-row block swap via DMA (DVE can't permute
                    # partitions), then a bf16 add.
                    uw = rpool.tile([128, 2, TC], bf16, tag="ruw")
                    ws = rpool.tile([128, TC], bf16, tag="rws")
                    nc.vector.tensor_mul(
                        out=uw, in0=ps.unsqueeze(1).broadcast_to((128, 2, TC)),
                        in1=t_cs[:, :, sl])
                    for b0 in range(0, 128, 32):
                        nc.gpsimd.dma_start(
                            out=ws[b0:b0 + 32, :],
                            in_=uw[b0 ^ 32:(b0 ^ 32) + 32, 1, :])
                    nc.vector.tensor_add(out=q, in0=uw[:, 0, :], in1=ws)

                def v_block(tt):
                    ps = mm_ps.tile([128, JV], f32, tag="mm", name="psv")
                    for ci in range(NCT):
                        nc.tensor.matmul(
                            ps,
                            lhsT=t_xT[:, ci, tt * 128:(tt + 1) * 128],
                            rhs=t_wv[:, ci, :],
                            start=(ci == 0), stop=(ci == NCT - 1))
                    nc.vector.tensor_copy(
                        out=vv[:, tt, :, 0:64],
                        in_=ps.rearrange("p (h c) -> p h c", h=4))

                # prefetch next chunk's cos/sin while this one computes;
                # qk order [0,2,1,3] completes head-pair 0's q AND k
                # first so its scores (and exp) start one rope earlier
                if tcn + 1 < NQC:
                    yield (lambda t=tcn + 1: cs_chunk(t))
                for jt in (0, 2, 1, 3):
                    yield (lambda j=jt: qk_block(j))
                for tt in range(4 * tcn, 4 * tcn + 4):
                    yield (lambda t=tt: v_block(t))

            def segment(tcn):
                for f in segment_blocks(tcn):
                    f()

            def attn_half(qc, hp, y_qc, filler=None, final=False,
                          quota=0):
                """scores+softmax+AV+normalize for q-chunk qc, head pair
                hp. AV for group g is emitted after ST of group g+2 (its
                exp has landed by then) so the PE never head-of-line
                blocks on an exp wait; `filler` supplies extra PE work
                (segments/proj) to densify the stream. The normalize uses
                a PE-free DRAM-broadcast (so the PE FIFO never waits on
                the DVE reciprocal); only the final half uses a broadcast
                matmul (PE is idle then) to cut the tail latency."""
                nkt = 4 * (qc + 1)
                qsl = slice(qc * TC, (qc + 1) * TC)
                pA = ppool.tile([128, NKT, TC], bf16, tag="pbuf")
                pB = ppool.tile([128, NKT, TC], bf16, tag="pbuf")
                yA = y_ps.tile([65, TC], f32, tag="yps")
                yB = y_ps.tile([65, TC], f32, tag="yps")

                def off_of(kt):
                    off = kt * 128 - qc * TC
                    return off if 0 <= off < TC else 0

                def av(kt):
                    off = off_of(kt)
                    for half, (yps, p) in enumerate(((yA, pA), (yB, pB))):
                        h = 2 * hp + half
                        nc.tensor.matmul(
                            yps[:, off:TC],
                            lhsT=t_v[:, kt, h * VGW:h * VGW + 65],
                            rhs=p[:, kt, off:TC],
                            start=(kt == 0), stop=(kt == nkt - 1))

                ngrp = (nkt + ST_G - 1) // ST_G
                consumed = 0
                for g in range(ngrp):
                    g0 = g * ST_G
                    gl = min(ST_G, nkt - g0)
                    stA = mm_ps.tile([128, ST_G, TC], f32, tag="mm")
                    stB = mm_ps.tile([128, ST_G, TC], f32, tag="mm")
                    for kg in range(gl):
                        kt = g0 + kg
                        ksl = slice(kt * 128, (kt + 1) * 128)
                        off = off_of(kt)
                        qsl_o = slice(qc * TC + off, (qc + 1) * TC)
                        nc.tensor.matmul(
                            stA[:, kg, off:TC],
                            lhsT=t_qkT[0:64, 2 + hp, ksl],
                            rhs=t_qkT[0:64, hp, qsl_o],
                            start=True, stop=True)
                        nc.tensor.matmul(
                            stB[:, kg, off:TC],
                            lhsT=t_qkT[64:128, 2 + hp, ksl],
                            rhs=t_qkT[64:128, hp, qsl_o],
                            start=True, stop=True)
                    # AV for the group 2 back, plus periodic filler to
                    # cover the PE-vs-ACT deficit
                    if g >= 2:
                        for kt in range((g - 2) * ST_G,
                                        (g - 2) * ST_G + ST_G):
                            av(kt)
                    # spread the filler budget evenly across groups so
                    # neither the PE nor ScalarE sees a long one-sided
                    # stretch (big end-of-chunk drains starve the exp)
                    if filler is not None:
                        want = -(-quota * (g + 1) // ngrp)  # ceil
                        while consumed < want:
                            f = next(filler, None)
                            if f is None:
                                break
                            f()
                            consumed += 1
                    nc.scalar.activation(
                        out=pA[:, g0:g0 + gl, :], in_=stA[:, 0:gl, :],
                        func=EXP, scale=0.125)
                    nc.scalar.activation(
                        out=pB[:, g0:g0 + gl, :], in_=stB[:, 0:gl, :],
                        func=EXP, scale=0.125)
                    # causal mask: only the 128-wide triangle block of
                    # diagonal k-tiles needs masking (prefix cols are
                    # skipped in the AV/score matmuls entirely)
                    for kg in range(gl):
                        kt = g0 + kg
                        off = kt * 128 - qc * TC
                        if 0 <= off < TC:
                            dsl = slice(off, off + 128)
                            nc.vector.tensor_mul(
                                out=pA[:, kt, dsl], in0=pA[:, kt, dsl],
                                in1=t_tri)
                            nc.vector.tensor_mul(
                                out=pB[:, kt, dsl], in0=pB[:, kt, dsl],
                                in1=t_tri)
                for kt in range(max(0, (ngrp - 2) * ST_G), nkt):
                    av(kt)
                # evacuate PSUM now: denominators to f32 rows, bodies to
                # bf16 SBUF; approx-reciprocal runs right behind on DVE
                d2 = npool.tile([1, 2, TC], f32, tag="d2")
                r2 = npool.tile([1, 2, TC], f32, tag="r2")
                ySb = npool.tile([64, 2, TC], bf16, tag="ysb")
                nc.vector.tensor_copy(out=d2[:, 0, :], in_=yA[64:65, :])
                nc.vector.tensor_copy(out=d2[:, 1, :], in_=yB[64:65, :])
                nc.vector.tensor_copy(out=ySb[:, 0, :], in_=yA[0:64, :])
                nc.vector.tensor_copy(out=ySb[:, 1, :], in_=yB[0:64, :])
                nc.vector.reciprocal_approx_fast(out=r2, in_=d2)
                if final:
                    for half in (0, 1):
                        rb_ps = mm_ps.tile([64, TC], f32, tag="mm",
                                           name="psrb")
                        nc.tensor.matmul(rb_ps, lhsT=t_one,
                                         rhs=r2[:, half, :],
                                         start=True, stop=True)
                        rb = rpool.tile([64, TC], f32, tag="r64")
                        nc.vector.tensor_copy(out=rb, in_=rb_ps)
                        nc.vector.tensor_mul(
                            out=y_qc[half * 64:(half + 1) * 64, hp, :],
                            in0=ySb[:, half, :], in1=rb)
                else:
                    rd = dram.tile([1, 2, TC], f32, tag="rd")
                    nc.gpsimd.dma_start(out=rd, in_=r2)
                    for half in (0, 1):
                        rb = rpool.tile([64, TC], f32, tag="r64")
                        nc.gpsimd.dma_start(
                            out=rb,
                            in_=rd[:, half, :].to_broadcast((64, TC)))
                        nc.vector.tensor_mul(
                            out=y_qc[half * 64:(half + 1) * 64, hp, :],
                            in0=ySb[:, half, :], in1=rb)

            def proj_blocks(qc, y_qc, final=False):
                for co in range(NCT):
                    def co_block(co=co):
                        ps = mm_ps.tile([128, TC], f32, tag="mm", name="psp")
                        for ci in range(2):
                            nc.tensor.matmul(
                                ps,
                                lhsT=t_wpr[:, ci, co * 128:(co + 1) * 128],
                                rhs=y_qc[:, ci, :],
                                start=(ci == 0), stop=(ci == 1))
                        o_sb = rpool.tile([128, TC], bf16, tag="osb")
                        # in the final chunk ScalarE is done with exp, so
                        # split the PSUM->SBUF casts across both engines
                        # to shorten the tail
                        if final and co % 2 == 1:
                            nc.scalar.copy(out=o_sb, in_=ps)
                        else:
                            nc.vector.tensor_copy(out=o_sb, in_=ps)
                        qs[co % 2].dma_start(
                            out=yT_out[co * 128:(co + 1) * 128,
                                       qc * TC:(qc + 1) * TC],
                            in_=o_sb)
                    yield co_block

            # interleave: segment(0), then attention chunk qc consumes
            # segment(qc+1) and proj(qc-1) blocks as PE filler while
            # ScalarE works through the exp volume
            y_qcs = [None] * NQC
            import itertools
            segment(0)
            for qc in range(NQC):
                y_qc = ypool.tile([128, 2, TC], bf16, tag="yqc")
                y_qcs[qc] = y_qc
                fills = []
                if qc + 1 < NQC:
                    fills.extend(segment_blocks(qc + 1))
                if qc >= 1:
                    fills.extend(proj_blocks(qc - 1, y_qcs[qc - 1]))
                nf = len(fills)
                fill = iter(fills)
                attn_half(qc, 0, y_qc, fill, quota=(nf + 1) // 2)
                attn_half(qc, 1, y_qc, fill, final=(qc == NQC - 1),
                          quota=nf)
                for f in fill:
                    f()
            for f in proj_blocks(NQC - 1, y_qcs[NQC - 1], final=True):
                f()

    nc.compile()
    return nc


def _prep_inputs(x, w_qkv, w_proj, freqs_cos, freqs_sin):
    bf = ml_dtypes.bfloat16
    cos = np.asarray(freqs_cos, np.float32)   # [T, 32]
    sin = np.asarray(freqs_sin, np.float32)
    # even/odd-split RoPE: within each head, q/k columns are permuted to
    # [d0,d2,..,d62, d1,d3,..,d63]; patterns are 32-row blocks
    cos_p = np.tile(cos.T, (4, 1))                             # [128, T]
    sin_p = np.tile(np.concatenate([sin.T, -sin.T], 0), (2, 1))
    cs = np.concatenate([cos_p, sin_p], axis=1).astype(bf)     # [128, 2T]
    eo = np.concatenate([np.arange(0, HD, 2), np.arange(1, HD, 2)])
    # causal triangle for the 128-wide diagonal block: keep iff col >= row
    kp = np.arange(128)
    tri = (kp[None, :] >= kp[:, None]).astype(bf)   # [row k, col j]: j >= k

    x = np.asarray(x, np.float32)
    w_qkv = np.asarray(w_qkv, np.float32)
    w_proj = np.asarray(w_proj, np.float32)
    in_maps = []
    # per-head even/odd column permutation for q and k blocks
    perm = np.concatenate([h * HD + eo for h in range(H)])
    wq_p = w_qkv[:, 0 * C:1 * C][:, perm]
    wk_p = w_qkv[:, 1 * C:2 * C][:, perm]
    for i in range(N_CORES):
        b, t = divmod(i, 4)
        jq = slice(t * JV, (t + 1) * JV)
        wq = wq_p[:, jq]
        wk = wk_p[:, jq]
        wv = w_qkv[:, 2 * C:3 * C][:, jq]
        in_maps.append({
            "xT": np.ascontiguousarray(x[b].T).astype(bf),
            "w_qk": np.concatenate([wq, wk], axis=1).astype(bf),
            "w_v": np.ascontiguousarray(wv).astype(bf),
            "w_pr": np.ascontiguousarray(w_proj[t * JV:(t + 1) * JV, :]).astype(bf),
            "cs": cs, "tri": tri,
        })
    return in_maps


def run(inputs, trace=False):
    from concourse import bass_utils
    if "nc" not in _CACHE:
        _CACHE["nc"] = _build()
    nc = _CACHE["nc"]
    in_maps = _prep_inputs(**inputs)
    res = bass_utils.run_bass_kernel_spmd(
        nc, in_maps, core_ids=list(range(N_CORES)), trace=trace)
    out = np.empty((B, T, C), np.float32)
    for b in range(B):
        acc = res.results[b * 4]["yT"].astype(np.float32)
        for t in range(1, 4):
            acc += res.results[b * 4 + t]["yT"]
        out[b] = acc.T
    return out, res


def kernel(**inputs):
    out, _ = run(inputs, trace=False)
    return out
